# revision 26
# baseline (speedup 1.0000x reference)
"""Trainium2 Bass kernel for nn_AutoRegressive_12128987644588.

6-layer post-norm transformer decoder (self-attn w/ prefix-causal mask,
cross-attn to packed embeddings, FFN), B=4, seq 865 (pad 896), D=1024,
16 heads x 64, FF=4096, final proj to 1024.

Sharding: 8 cores = 4 batches x 2 sequence halves of 448 tokens.
Per layer the two cores of a batch AllGather their x^T halves (the only
collective); K/V projections are computed over the full sequence on both
cores (duplicate compute, no other comm). Activations live transposed
[feature, token] in SBUF so every GEMM is natural (lhsT = W^T chunk,
rhs = x^T chunk) and all out-feature biases are per-partition. x tiles
are updated in place (residual adds and LayerNorm write back).

Attention: scores are computed transposed S^T[tk, tq] per head via
K=64 matmuls. Two heads of a pair land in one 2-bank PSUM supertile
([128, 1024], head A in cols 0:448, head B in 512:960) so a single
ScalarE exp instruction covers both heads (the per-instruction ~300ns
ACT overhead is paid once per pair instead of once per head). PV runs
one k-tile behind the S/exp stream so the PE never waits on an exp it
just issued. V_aug carries a ones column so the softmax denominator
falls out of the same PV matmul; denominators of a pair are broadcast
with one K=2 matmul.

The PE is kept dense through the exp-bound attention phases and the
LayerNorm stat chains by interleaving independent GEMM work: each
projection is emitted as a generator of PSUM-group chunks, and the
cross-attention K/V projection of the current layer (computed from the
layer-invariant memory embeddings, resident in SBUF) is pulled chunk by
chunk into the AllGather gap, the SA attention stream, and the LN
windows. This avoids PE idle gaps that would re-throttle the HAM clock.

Embedding gather/pack/positional encodings are integer-indexed data
staging done on host; all FLOPs run on device.
"""
from collections import deque

import numpy as np

import concourse.bass as bass
import concourse.mybir as mybir
import concourse.tile as tile
from concourse import bacc, bass_utils

F32 = mybir.dt.float32
F32R = mybir.dt.float32r
BF16 = mybir.dt.float16  # fp16: FWL-eligible, 10-bit mantissa

B, D, H, HD, FF, L = 4, 1024, 16, 64, 4096, 6
TT, TA, ENR = 128, 512, 225
SEQ = TT + TA + ENR            # 865
TPAD = 896                     # 7 * 128
TH = 448                       # per-core half (padded)
PREFIX = TT + TA               # 640 = 5 * 128
NKT = TPAD // 128              # 7 key tiles
ND = D // 128                  # 8 feature tiles
VOCAB = 1024
EPS = 1e-5
NEG = -1e9
PW = 960                       # packed pair width: 448 | 64 gap | 448


# ---------------------------------------------------------------- host side

def sinusoidal_pe(T, d):
    pos = np.arange(T, dtype=np.float32)[:, None]
    div = np.exp(np.arange(0, d, 2, dtype=np.float32) * (-np.log(10000.0) / d))
    pe = np.zeros((T, d), dtype=np.float32)
    pe[:, 0::2] = np.sin(pos * div)
    pe[:, 1::2] = np.cos(pos * div)
    return pe


def host_embed(text, audio, enrolled_audio, text_len, audio_len,
               text_emb, audio_emb):
    """Replicates reference embed+pack. Returns [B, TPAD, D] f32 (pad zeros)."""
    te = text_emb[text] + sinusoidal_pe(TT, D)[None]        # [B,TT,D]
    ae = audio_emb[audio] + sinusoidal_pe(TA, D)[None]      # [B,TA,D]
    ee = audio_emb[enrolled_audio] + sinusoidal_pe(ENR, D)[None]
    out = np.zeros((B, TPAD, D), dtype=np.float32)
    for b in range(B):
        tl, al = int(text_len[b]), int(audio_len[b])
        out[b, :tl] = te[b, :tl]
        out[b, tl:tl + al] = ae[b, :al]
        out[b, tl + al:tl + al + ENR] = ee[b]
    return out


def host_masks(half):
    """Additive mask for SA key tiles 5,6 transposed: [256, TH]."""
    k = np.arange(PREFIX, PREFIX + 256)[:, None]            # 640..895
    q = half * TH + np.arange(TH)[None, :]
    blocked = (k > q) | (k >= SEQ)
    return np.where(blocked, NEG, 0.0).astype(np.float32)


def host_maskrep(half):
    """[2, 128, PW] pair-replicated masks (cols 0:448 and 512:960).
    fp16, so the blocked sentinel is -60000 (exp underflows to 0)."""
    m = host_masks(half)                                    # [256, TH]
    m = np.maximum(m, -60000.0)
    out = np.zeros((2, 128, PW), dtype=np.float16)
    for i in range(2):
        out[i, :, 0:TH] = m[i * 128:(i + 1) * 128]
        out[i, :, 512:512 + TH] = m[i * 128:(i + 1) * 128]
    return out


def host_kvalid6():
    k = PREFIX + 128 + np.arange(128)                       # 768..895
    return np.where(k < SEQ, 0.0, NEG).astype(np.float32)[:, None]


# ---------------------------------------------------------------- builder

def build_kernel(n_layers=L, skip_bv=False):
    nc = bacc.Bacc("TRN2", target_bir_lowering=False, debug=False,
                   num_devices=8)

    def din(name, shape, dt=F32R):
        return nc.dram_tensor(name, shape, dt, kind="ExternalInput")

    xT0_d = din("xT0", [D, TH])
    memT_d = din("memT", [D, TPAD], BF16)
    maskrep_d = din("maskrep", [2, 128, PW], BF16)
    kval6_d = din("kval6", [128, 1], F32)
    ones_col_d = din("ones_col", [128, 1])
    ones_r128_d = din("ones_r128", [1, 128])
    vones_d = din("vones", [128, H], BF16)
    k2sel_d = din("k2sel", [2, 128])
    neg_r448_d = din("neg_r448", [1, TH])

    sa_inT_d = din("sa_inT", [L, D, 3 * D], BF16)
    sa_outT_d = din("sa_outT", [L, D, D], BF16)
    ca_inT_d = din("ca_inT", [L, D, 3 * D], BF16)
    ca_outT_d = din("ca_outT", [L, D, D], BF16)
    ff1T_d = din("ff1T", [L, D, FF], BF16)
    ff2T_d = din("ff2T", [L, FF, D], BF16)
    outT_d = din("outT", [D, VOCAB], BF16)

    sa_inb_d = din("sa_inb", [L, 3 * D], F32)
    sa_outb_d = din("sa_outb", [L, D], F32)
    ca_inb_d = din("ca_inb", [L, 3 * D], F32)
    ca_outb_d = din("ca_outb", [L, D], F32)
    ff1b_d = din("ff1b", [L, FF], F32)
    ff2b_d = din("ff2b", [L, D], F32)
    outb_d = din("outb", [VOCAB], F32)
    lnw_d = [din(f"ln{i}w", [L, D], F32) for i in (1, 2, 3)]
    lnb_d = [din(f"ln{i}b", [L, D], F32) for i in (1, 2, 3)]

    yT_d = nc.dram_tensor("yT", [VOCAB, TH], F32, kind="ExternalOutput")

    uid = [0]

    def nm(p):
        uid[0] += 1
        return f"{p}_{uid[0]}"

    with tile.TileContext(nc) as tc:
        with (
            nc.allow_low_precision(reason="f32r compute; tol 2e-2"),
            tc.tile_pool(name="const", bufs=1) as constp,
            tc.tile_pool(name="xpool", bufs=8) as xpool,

            tc.tile_pool(name="tmpp", bufs=4) as tmpp,
            tc.tile_pool(name="rows", bufs=2) as rowp,
            tc.tile_pool(name="statp", bufs=2) as statp,
            tc.tile_pool(name="biasp", bufs=12) as biasp,
            tc.tile_pool(name="wglob", bufs=28) as wglob,
            tc.tile_pool(name="gemmps", bufs=2, space="PSUM") as gemmps,
            tc.tile_pool(name="dram", bufs=2, space="DRAM") as dramp,
        ):
            # ---- constants
            ones_col = constp.tile([128, 1], F32R, name="ones_col")
            ones_r128 = constp.tile([1, 128], F32R, name="ones_r128")
            vones = constp.tile([128, H], BF16, name="vones")
            kval6 = constp.tile([128, 1], F32, name="kval6")
            mask5 = constp.tile([128, PW], BF16, name="mask5")
            mask6 = constp.tile([128, PW], BF16, name="mask6")
            k2sel_a = constp.tile([1, 128], F32R, name="k2sel_a")
            k2sel_b = constp.tile([1, 128], F32R, name="k2sel_b")
            lnrhs = constp.tile([2, TH], F32R, name="lnrhs")
            nc.sync.dma_start(out=ones_col[:], in_=ones_col_d.ap())
            nc.sync.dma_start(out=ones_r128[:], in_=ones_r128_d.ap())
            nc.sync.dma_start(out=vones[:], in_=vones_d.ap())
            nc.sync.dma_start(out=kval6[:], in_=kval6_d.ap())
            nc.sync.dma_start(out=mask5[:], in_=maskrep_d.ap()[0])
            nc.sync.dma_start(out=mask6[:], in_=maskrep_d.ap()[1])
            nc.sync.dma_start(out=k2sel_a[:], in_=k2sel_d.ap()[0:1, :])
            nc.sync.dma_start(out=k2sel_b[:], in_=k2sel_d.ap()[1:2, :])
            nc.sync.dma_start(out=lnrhs[1:2, :], in_=neg_r448_d.ap())
            eps_tile = constp.tile([1, 1], F32, name="eps_tile")
            nc.vector.memset(eps_tile[:], EPS)

            # ---- x tiles: fixed, updated in place through the whole net
            # (f32r master) plus bf16 shadows used as GEMM moving operands
            x_cur = []
            xb16 = []
            for t in range(ND):
                xt = xpool.tile([128, TH], F32R, name=nm("x"), tag="x")
                nc.sync.dma_start(out=xt[:],
                                  in_=xT0_d.ap()[t * 128:(t + 1) * 128, :])
                x_cur.append(xt)
                xb = xpool.tile([128, TH], BF16, name=nm("xb"), tag="xb")
                nc.vector.tensor_copy(xb[:], xt[:])
                xb16.append(xb)

            # ------------------------------------------- filler machinery
            pending = deque()

            def pull(n=1):
                k = 0
                while k < n and pending:
                    try:
                        next(pending[0])
                        k += 1
                    except StopIteration:
                        pending.popleft()

            def drain(g):
                for _ in g:
                    pass

            def drain_pending():
                while pending:
                    pull(1)

            # ---------------------------------------------------- helpers
            def load_bias_col(src_1d_ap, n, name):
                t = biasp.tile([128, n], F32, name=nm(name), tag="bcol")
                nc.sync.dma_start(
                    out=t[:], in_=src_1d_ap.rearrange("(c p) -> p c", p=128))
                return t

            def load_row(src_1d_ap, n, name):
                t = rowp.tile([1, n], F32R, name=nm(name), tag="row")
                nc.sync.dma_start(
                    out=t[:],
                    in_=src_1d_ap.rearrange("(a f) -> a f", a=1).bitcast(F32R))
                return t

            def gemm_chunks(wT2d, rhs_tiles, nout, evict, fdim=TH):
                """Generator: out^T[nout, fdim] = W @ rhs, yielding after
                each PSUM-group (one 128-row output tile)."""
                nk = len(rhs_tiles)
                for n0 in range(0, nout, 512):
                    w = min(512, nout - n0)
                    wts = []
                    for k in range(nk):
                        wt = wglob.tile([128, w], BF16, name=nm("w"),
                                        tag="w", bufs=16)
                        nc.sync.dma_start(
                            out=wt[:],
                            in_=wT2d[k * 128:(k + 1) * 128, n0:n0 + w])
                        wts.append(wt)
                    for m0 in range(0, w, 128):
                        ps = gemmps.tile([128, 512], F32, name=nm("pg"),
                                         tag="pg", bufs=2)
                        for k in range(nk):
                            nc.tensor.matmul(
                                ps[:, 0:fdim], wts[k][:, m0:m0 + 128],
                                rhs_tiles[k][:, :fdim],
                                start=(k == 0), stop=(k == nk - 1))
                        evict(n0 + m0, ps[:, 0:fdim])
                        yield

            def kv_chunks(inT2d, inb1d, src_tiles, kt, va):
                """Generator: fills kt [8 x (128, TPAD)] and
                va [7 x (128, H*65)] from src_tiles (8 x [128, TPAD])."""
                bk_col = load_bias_col(inb1d[D:2 * D], ND, "bk")
                bv_row = None if skip_bv else load_row(
                    inb1d[2 * D:3 * D], D, "bv")
                for t in range(NKT):
                    nc.sync.dma_start(
                        out=va[t][:].rearrange("p (h e) -> p h e", e=65)
                        [:, :, 64:65],
                        in_=vones_d.ap())
                if True:
                    for f0 in (0, TH):
                        for n0 in (0, 512):
                            wts = []
                            for k in range(ND):
                                wt = wglob.tile([128, 512], BF16,
                                                name=nm("w"), tag="w",
                                                bufs=16)
                                nc.sync.dma_start(
                                    out=wt[:],
                                    in_=inT2d[k * 128:(k + 1) * 128,
                                              D + n0:D + n0 + 512])
                                wts.append(wt)
                            for m0 in range(0, 512, 128):
                                ps = gemmps.tile([128, 512], F32,
                                                 name=nm("pk"), tag="pg",
                                                 bufs=2)
                                for k in range(ND):
                                    nc.tensor.matmul(
                                        ps[:, 0:TH], wts[k][:, m0:m0 + 128],
                                        src_tiles[k][:, f0:f0 + TH],
                                        start=(k == 0), stop=(k == ND - 1))
                                nc.vector.tensor_scalar_add(
                                    kt[(n0 + m0) // 128][:, f0:f0 + TH],
                                    ps[:, 0:TH],
                                    bk_col[:, (n0 + m0) // 128:
                                           (n0 + m0) // 128 + 1])
                                yield
                    for c0 in (0, 512):
                        wts = []
                        for k in range(ND):
                            wt = wglob.tile([128, 512], BF16, name=nm("wv"),
                                            tag="w", bufs=16)
                            nc.sync.dma_start(
                                out=wt[:],
                                in_=inT2d[k * 128:(k + 1) * 128,
                                          2 * D + c0:2 * D + c0 + 512])
                            wts.append(wt)
                        for t in range(NKT):
                            ps = gemmps.tile([128, 512], F32,
                                             name=nm("pv"), tag="pg",
                                             bufs=2)
                            for k in range(ND):
                                nc.tensor.matmul(
                                    ps[:],
                                    src_tiles[k][:, t * 128:(t + 1) * 128],
                                    wts[k][:], start=(k == 0),
                                    stop=(bv_row is None and k == ND - 1))
                            if bv_row is not None:
                                nc.tensor.matmul(ps[:], ones_r128[:, :128],
                                                 bv_row[:, c0:c0 + 512],
                                                 start=False, stop=True)
                            nc.vector.tensor_copy(
                                va[t][:].rearrange("p (h e) -> p h e", e=65)
                                [:, c0 // 64:c0 // 64 + 8, 0:64],
                                ps[:].rearrange("p (h e) -> p h e", e=64))
                            yield

            def layer_norm(x_tiles, w_col, w_row):
                """In-place post-norm LN over the feature (partition) dim."""
                with tc.tile_pool(name=nm("lnps"), bufs=2, space="PSUM") as lps:
                    mu_ps = lps.tile([1, TH], F32, name=nm("mups"), bufs=1)
                    s2_ps = lps.tile([1, TH], F32, name=nm("s2ps"), bufs=1)
                    for t in range(ND):
                        nc.tensor.matmul(mu_ps[:], ones_col[:],
                                         x_tiles[t][:],
                                         start=(t == 0), stop=(t == ND - 1))
                    for t in range(ND):
                        sq = tmpp.tile([128, TH], F32R, name=nm("sq"),
                                       tag="tmp")
                        nc.scalar.square(sq[:], x_tiles[t][:])
                        nc.tensor.matmul(s2_ps[:], ones_col[:], sq[:],
                                         start=(t == 0), stop=(t == ND - 1))
                    pull(2)
                    muex = statp.tile([1, 2 * TH], F32, name=nm("muex"),
                                      tag="st2", bufs=1)
                    mu = muex[0:1, 0:TH]
                    ex2 = muex[0:1, TH:2 * TH]
                    nc.scalar.activation(mu, mu_ps[:],
                                         mybir.ActivationFunctionType.Copy,
                                         scale=1.0 / D)
                    nc.scalar.activation(ex2, s2_ps[:],
                                         mybir.ActivationFunctionType.Copy,
                                         scale=1.0 / D)
                    var = statp.tile([1, TH], F32, name=nm("var"), tag="st")
                    nc.vector.tensor_tensor(var[:], mu, mu,
                                            mybir.AluOpType.mult)
                    nc.vector.tensor_tensor(var[:], ex2, var[:],
                                            mybir.AluOpType.subtract)
                    sd = statp.tile([1, TH], F32R, name=nm("sd"), tag="st")
                    nc.scalar.activation(sd[:], var[:],
                                         mybir.ActivationFunctionType.Sqrt,
                                         bias=eps_tile[:])
                    sdb_ps = lps.tile([128, TH], F32, name=nm("sdb"), bufs=1)
                    nc.tensor.matmul(sdb_ps[:], ones_r128[:], sd[:],
                                     start=True, stop=True)
                    pull(2)
                    rs_b = tmpp.tile([128, TH], F32, name=nm("rsb"),
                                     tag="rb", bufs=2)
                    nc.vector.reciprocal_approx_fast(out=rs_b[:],
                                                     in_=sdb_ps[:])
                    nc.vector.tensor_tensor(lnrhs[0:1, :], mu, rs_b[0:1, :],
                                            mybir.AluOpType.mult)
                    for t in range(ND):
                        aux = lps.tile([128, TH], F32, name=nm("aux"),
                                       tag="lnaux", bufs=2)
                        nc.tensor.matmul(aux[:],
                                         w_row[:, t * 128:(t + 1) * 128],
                                         lnrhs[:], start=True, stop=True)
                        t1 = tmpp.tile([128, TH], F32R, name=nm("t1"),
                                       tag="tmp")
                        nc.vector.tensor_tensor(t1[:], x_tiles[t][:],
                                                rs_b[:],
                                                mybir.AluOpType.mult)
                        nc.vector.scalar_tensor_tensor(
                            x_tiles[t][:], t1[:], w_col[:, t:t + 1], aux[:],
                            mybir.AluOpType.mult, mybir.AluOpType.subtract)
                        nc.vector.tensor_copy(xb16[t][:], x_tiles[t][:])

            def attention(pp, q_tiles, kt_tiles, vaug_tiles, masks, kval):
                """Returns attnT tiles (8 x [128, TH]) in pool pp.
                Pair-packed S/exp + PV one k-tile behind; pulls one filler
                chunk per k-tile step to keep the PE dense."""
                at = [pp.tile([128, TH], BF16, name=nm("at"), tag="attnT",
                              bufs=8) for _ in range(ND)]
                with (
                    tc.tile_pool(name=nm("aps"), bufs=1, space="PSUM") as sps,
                    tc.tile_pool(name=nm("ops"), bufs=4, space="PSUM") as ops,
                ):
                    for h0 in range(0, H, 4):
                        quad = (h0, h0 + 1, h0 + 2, h0 + 3)
                        o_ps = {}
                        for hh in quad:
                            o_ps[hh] = ops.tile([65, TH], F32,
                                                name=nm("ops"), tag="po",
                                                bufs=4)
                        pbs = {}
                        for t in range(NKT + 1):
                            if t < NKT:
                                for pi in (0, 1):
                                    ha = h0 + 2 * pi
                                    ti = ha // 2
                                    ps = sps.tile([128, 1024], F32,
                                                  name=nm("sps"), tag="sst",
                                                  bufs=1)
                                    nc.tensor.matmul(
                                        ps[:, 0:TH],
                                        kt_tiles[ti][0:64,
                                                     t * 128:(t + 1) * 128],
                                        q_tiles[ti][0:64, :],
                                        start=True, stop=True)
                                    nc.tensor.matmul(
                                        ps[:, 512:512 + TH],
                                        kt_tiles[ti][64:128,
                                                     t * 128:(t + 1) * 128],
                                        q_tiles[ti][64:128, :],
                                        start=True, stop=True)
                                    pb = tmpp.tile([128, PW], BF16,
                                                   name=nm("p"), tag="pexp",
                                                   bufs=3)
                                    if masks is not None and t >= 5:
                                        tm = tmpp.tile([128, PW], F32R,
                                                       name=nm("sm"),
                                                       tag="ptmp", bufs=1)
                                        nc.vector.tensor_tensor(
                                            tm[:], ps[:, 0:PW],
                                            masks[t - 5][:],
                                            mybir.AluOpType.add)
                                        nc.scalar.activation(
                                            pb[:], tm[:],
                                            mybir.ActivationFunctionType.Exp)
                                    elif kval is not None and t == NKT - 1:
                                        nc.scalar.activation(
                                            pb[:], ps[:, 0:PW],
                                            mybir.ActivationFunctionType.Exp,
                                            bias=kval[:])
                                    else:
                                        nc.scalar.activation(
                                            pb[:], ps[:, 0:PW],
                                            mybir.ActivationFunctionType.Exp)
                                    pbs[(pi, t)] = pb
                            if t > 0:
                                for pi in (0, 1):
                                    ha = h0 + 2 * pi
                                    pb = pbs.pop((pi, t - 1))
                                    for s, hh in ((0, ha), (512, ha + 1)):
                                        nc.tensor.matmul(
                                            o_ps[hh][:],
                                            vaug_tiles[t - 1][:].rearrange(
                                                "p (h e) -> p h e",
                                                e=65)[:, hh, :],
                                            pb[:, s:s + TH],
                                            start=(t - 1 == 0),
                                            stop=(t - 1 == NKT - 1))
                            pull(1)
                        for pi in (0, 1):
                            ha = h0 + 2 * pi
                            dena = statp.tile([1, TH], F32R, name=nm("dna"),
                                              tag="stda", bufs=2)
                            denb = statp.tile([1, TH], F32R, name=nm("dnb"),
                                              tag="stdb", bufs=2)
                            nc.vector.tensor_copy(dena[:],
                                                  o_ps[ha][64:65, :])
                            nc.vector.tensor_copy(denb[:],
                                                  o_ps[ha + 1][64:65, :])
                            r_ps = sps.tile([128, 1024], F32, name=nm("rps"),
                                            tag="sst", bufs=1)
                            nc.tensor.matmul(r_ps[:, 0:TH],
                                             k2sel_a[:], dena[:],
                                             start=True, stop=False)
                            nc.tensor.matmul(r_ps[:, 0:TH],
                                             k2sel_b[:], denb[:],
                                             start=False, stop=True)
                            rb = tmpp.tile([128, TH], F32, name=nm("rb"),
                                           tag="rb", bufs=2)
                            nc.vector.reciprocal_approx_fast(
                                out=rb[:], in_=r_ps[:, 0:TH])
                            ti = ha // 2
                            nc.vector.tensor_tensor(
                                at[ti][0:64, :], o_ps[ha][0:64, :],
                                rb[0:64, :], mybir.AluOpType.mult)
                            nc.vector.tensor_tensor(
                                at[ti][64:128, :], o_ps[ha + 1][0:64, :],
                                rb[64:128, :], mybir.AluOpType.mult)
                return at

            def qproj_gen(pp, inT2d, inb1d):
                q_t = [pp.tile([128, TH], BF16, name=nm("q"), tag="q",
                               bufs=8) for _ in range(ND)]
                bq_col = load_bias_col(inb1d[0:D], ND, "bq")

                def ev_q(n0, ps):
                    nc.vector.tensor_scalar_add(
                        q_t[n0 // 128][:], ps,
                        bq_col[:, n0 // 128:n0 // 128 + 1])
                return q_t, gemm_chunks(inT2d[:, 0:D], xb16, D, ev_q)

            def out_proj(wT2d, b1d, at):
                bo_col = load_bias_col(b1d, ND, "bo")

                def ev_o(n0, ps):
                    t = n0 // 128
                    nc.vector.scalar_tensor_tensor(
                        x_cur[t][:], ps, bo_col[:, t:t + 1], x_cur[t][:],
                        mybir.AluOpType.add, mybir.AluOpType.add)
                drain(gemm_chunks(wT2d, at, D, ev_o))

            def do_ln(idx, l):
                lwb = rowp.tile([2, D], F32R, name=nm(f"ln{idx}wb"),
                                tag="row")
                nc.sync.dma_start(
                    out=lwb[0:1, :],
                    in_=lnw_d[idx].ap()[l].rearrange(
                        "(a f) -> a f", a=1).bitcast(F32R))
                nc.sync.dma_start(
                    out=lwb[1:2, :],
                    in_=lnb_d[idx].ap()[l].rearrange(
                        "(a f) -> a f", a=1).bitcast(F32R))
                lwc = load_bias_col(lnw_d[idx].ap()[l], ND, f"ln{idx}wc")
                layer_norm(x_cur, lwc, lwb)

            # ---------------------------------------------------- layers
            for l in range(n_layers):
                # CA K/V of this layer: computed from the (re-loaded) memory
                # embeddings, consumed as filler during AG gap / SA
                # attention / LNs. Lives in its own pool spanning the layer.
                kvp_ctx = tc.tile_pool(name=nm("kvl"), bufs=2)
                kvp = kvp_ctx.__enter__()
                memt = [kvp.tile([128, TPAD], BF16, name=nm("memt"),
                                 tag="mem", bufs=ND) for _ in range(ND)]
                for t in range(ND):
                    nc.sync.dma_start(
                        out=memt[t][:],
                        in_=memT_d.ap()[t * 128:(t + 1) * 128, :])
                kt_ca = [kvp.tile([128, TPAD], BF16, name=nm("ktca"),
                                  tag="ktca", bufs=ND) for _ in range(ND)]
                va_ca = [kvp.tile([128, H * 65], BF16, name=nm("vaca"),
                                  tag="vaca", bufs=NKT) for _ in range(NKT)]
                pending.append(kv_chunks(ca_inT_d.ap()[l], ca_inb_d.ap()[l],
                                         memt, kt_ca, va_ca))

                ag_in = dramp.tile([D, TH], BF16, name=nm("agin"), tag="agi")
                ag_out = dramp.tile([2 * D, TH], BF16, name=nm("agout"),
                                    tag="ago")
                for t in range(ND):
                    nc.sync.dma_start(
                        out=ag_in[t * 128:(t + 1) * 128, :], in_=xb16[t][:])
                nc.gpsimd.collective_compute(
                    "AllGather", mybir.AluOpType.bypass,
                    replica_groups=[[0, 1], [2, 3], [4, 5], [6, 7]],
                    ins=[ag_in[:].opt()], outs=[ag_out[:].opt()])

                with tc.tile_pool(name=nm("attl"), bufs=2) as attl:
                    # fill the AllGather gap: SA Q proj (own half) + CA K/V
                    q_sa, q_gen = qproj_gen(attl, sa_inT_d.ap()[l],
                                            sa_inb_d.ap()[l])
                    drain(q_gen)
                    pull(14)

                    # SA K/V over the gathered full sequence
                    xfull = [attl.tile([128, TPAD], BF16, name=nm("xf"),
                                       tag="xfull", bufs=8)
                             for _ in range(ND)]
                    for t in range(ND):
                        nc.sync.dma_start(
                            out=xfull[t][:, 0:TH],
                            in_=ag_out[t * 128:(t + 1) * 128, :])
                        nc.sync.dma_start(
                            out=xfull[t][:, TH:TPAD],
                            in_=ag_out[D + t * 128:D + (t + 1) * 128, :])
                    kt_sa = [attl.tile([128, TPAD], BF16, name=nm("ktsa"),
                                       tag="ktsa", bufs=ND)
                             for _ in range(ND)]
                    va_sa = [attl.tile([128, H * 65], BF16, name=nm("vasa"),
                                       tag="vasa", bufs=NKT)
                             for _ in range(NKT)]
                    drain(kv_chunks(sa_inT_d.ap()[l], sa_inb_d.ap()[l],
                                    xfull, kt_sa, va_sa))

                    at = attention(attl, q_sa, kt_sa, va_sa,
                                   (mask5, mask6), None)
                    out_proj(sa_outT_d.ap()[l], sa_outb_d.ap()[l], at)
                    do_ln(0, l)

                q_ca, q_gen = qproj_gen(kvp, ca_inT_d.ap()[l],
                                        ca_inb_d.ap()[l])
                drain(q_gen)
                drain_pending()
                at = attention(kvp, q_ca, kt_ca, va_ca, None, kval6)
                out_proj(ca_outT_d.ap()[l], ca_outb_d.ap()[l], at)
                do_ln(1, l)

                # ================= FFN =================
                with tc.tile_pool(name=nm("ffl"), bufs=2) as ffp:
                    ht = [ffp.tile([128, TH], BF16, name=nm("h"), tag="h",
                                   bufs=FF // 128) for _ in range(FF // 128)]
                    b1_col = load_bias_col(ff1b_d.ap()[l], FF // 128, "b1")

                    def ev_h(n0, ps):
                        t = n0 // 128
                        nc.vector.tensor_scalar(
                            ht[t][:], ps, b1_col[:, t:t + 1], 0.0,
                            mybir.AluOpType.add, mybir.AluOpType.max)
                    drain(gemm_chunks(ff1T_d.ap()[l], xb16, FF, ev_h))

                    b2_col = load_bias_col(ff2b_d.ap()[l], ND, "b2")

                    def ev_f(n0, ps):
                        # first half adds the bias, second half adds plain
                        t = n0 // 128
                        nc.vector.scalar_tensor_tensor(
                            x_cur[t][:], ps, b2_col[:, t:t + 1], x_cur[t][:],
                            mybir.AluOpType.add, mybir.AluOpType.add)

                    def ev_f2(n0, ps):
                        t = n0 // 128
                        nc.vector.tensor_tensor(
                            x_cur[t][:], ps, x_cur[t][:],
                            mybir.AluOpType.add)
                    drain(gemm_chunks(ff2T_d.ap()[l][0:FF // 2, :], ht[:16],
                                      D, ev_f))
                    drain(gemm_chunks(ff2T_d.ap()[l][FF // 2:FF, :], ht[16:],
                                      D, ev_f2))
                    do_ln(2, l)
                kvp_ctx.__exit__(None, None, None)

            # ---- final projection
            ob_col = load_bias_col(outb_d.ap(), VOCAB // 128, "ob")

            def ev_y(n0, ps):
                y = tmpp.tile([128, TH], F32, name=nm("y"), tag="tmp")
                nc.vector.tensor_scalar_add(
                    y[:], ps, ob_col[:, n0 // 128:n0 // 128 + 1])
                nc.sync.dma_start(out=yT_d.ap()[n0:n0 + 128, :], in_=y[:])
            drain(gemm_chunks(outT_d.ap(), xb16, VOCAB, ev_y))

    nc.compile()
    return nc


# ---------------------------------------------------------------- wrapper

def prep_in_maps(inputs):
    f32 = lambda a: np.ascontiguousarray(np.asarray(a, dtype=np.float32))
    embed = host_embed(
        np.asarray(inputs["text"]), np.asarray(inputs["audio"]),
        np.asarray(inputs["enrolled_audio"]),
        np.asarray(inputs["text_len_batch"]),
        np.asarray(inputs["audio_len_batch"]),
        f32(inputs["text_emb"]), f32(inputs["audio_emb"]))
    embT = np.ascontiguousarray(embed.transpose(0, 2, 1))   # [B, D, TPAD]

    bf = lambda a: np.ascontiguousarray(a.astype(np.float16))
    tr = lambda a: np.ascontiguousarray(
        np.asarray(a, dtype=np.float32).transpose(0, 2, 1))
    sa_inT = tr(inputs["sa_in_w"])      # [L, D, 3D]
    ca_inT = tr(inputs["ca_in_w"])
    sa_inT[:, :, :D] *= 0.125           # fold 1/sqrt(hd) into Q
    ca_inT[:, :, :D] *= 0.125
    sa_inb = f32(inputs["sa_in_b"]).copy()
    ca_inb = f32(inputs["ca_in_b"]).copy()
    sa_inb[:, :D] *= 0.125
    ca_inb[:, :D] *= 0.125

    shared = dict(
        kval6=host_kvalid6(),
        ones_col=np.ones((128, 1), np.float32),
        ones_r128=np.ones((1, 128), np.float32),
        vones=np.ones((128, H), np.float16),
        neg_r448=np.full((1, 448), -1.0, np.float32),
        k2sel=np.concatenate([
            np.concatenate([np.ones((1, 64)), np.zeros((1, 64))], 1),
            np.concatenate([np.zeros((1, 64)), np.ones((1, 64))], 1),
        ]).astype(np.float32),
        sa_inT=bf(sa_inT), sa_outT=bf(tr(inputs["sa_out_w"])),
        ca_inT=bf(ca_inT), ca_outT=bf(tr(inputs["ca_out_w"])),
        ff1T=bf(tr(inputs["ff1_w"])), ff2T=bf(tr(inputs["ff2_w"])),
        outT=bf(np.ascontiguousarray(f32(inputs["out_w"]).T)),
        sa_inb=sa_inb, sa_outb=f32(inputs["sa_out_b"]),
        ca_inb=ca_inb, ca_outb=f32(inputs["ca_out_b"]),
        ff1b=f32(inputs["ff1_b"]), ff2b=f32(inputs["ff2_b"]),
        outb=f32(inputs["out_b"]),
        ln1w=f32(inputs["ln1_w"]), ln1b=f32(inputs["ln1_b"]),
        ln2w=f32(inputs["ln2_w"]), ln2b=f32(inputs["ln2_b"]),
        ln3w=f32(inputs["ln3_w"]), ln3b=f32(inputs["ln3_b"]),
    )
    in_maps = []
    for c in range(8):
        bb, hh = c // 2, c % 2
        m = dict(shared)
        m["xT0"] = np.ascontiguousarray(embT[bb][:, hh * TH:(hh + 1) * TH])
        m["memT"] = bf(embT[bb])
        m["maskrep"] = host_maskrep(hh)
        in_maps.append(m)
    return in_maps


_NC_CACHE = {}


def run(inputs, n_layers=L, trace=False):
    skip_bv = (not np.any(np.asarray(inputs["sa_in_b"])[:, 2 * D:])
               and not np.any(np.asarray(inputs["ca_in_b"])[:, 2 * D:]))
    key = (n_layers, skip_bv)
    if key not in _NC_CACHE:
        _NC_CACHE[key] = build_kernel(n_layers, skip_bv)
    nc = _NC_CACHE[key]
    in_maps = prep_in_maps(inputs)
    res = bass_utils.run_bass_kernel_spmd(
        nc, in_maps, core_ids=list(range(8)), trace=trace)
    out = np.zeros((B, SEQ, VOCAB), dtype=np.float32)
    for c in range(8):
        bb, hh = c // 2, c % 2
        cols = TH if hh == 0 else SEQ - TH
        out[bb, hh * TH:hh * TH + cols, :] = \
            res.results[c]["yT"][:, :cols].T
    return out, res


def kernel(**inputs):
    out, _ = run(inputs)
    return out


# revision 28
# speedup vs baseline: 1.2769x; 1.2769x over previous
"""Trainium2 Bass kernel for nn_AutoRegressive_12128987644588.

6-layer post-norm transformer decoder (self-attn w/ prefix-causal mask,
cross-attn to packed embeddings, FFN), B=4, seq 865 (pad 896), D=1024,
16 heads x 64, FF=4096, final proj to 1024.

Sharding: 8 cores = 4 batches x 2 sequence halves of 448 tokens.
Per layer the two cores of a batch AllGather their x^T halves (the only
collective); K/V projections are computed over the full sequence on both
cores (duplicate compute, no other comm). Activations live transposed
[feature, token] in SBUF so every GEMM is natural (lhsT = W^T chunk,
rhs = x^T chunk) and all out-feature biases are per-partition. x tiles
are updated in place (residual adds and LayerNorm write back).

Attention: scores are computed transposed S^T[tk, tq] per head via
K=64 matmuls. Two heads of a pair land in one 2-bank PSUM supertile
([128, 1024], head A in cols 0:448, head B in 512:960) so a single
ScalarE exp instruction covers both heads (the per-instruction ~300ns
ACT overhead is paid once per pair instead of once per head). PV runs
one k-tile behind the S/exp stream so the PE never waits on an exp it
just issued. V_aug carries a ones column so the softmax denominator
falls out of the same PV matmul; denominators of a pair are broadcast
with one K=2 matmul.

The PE is kept dense through the exp-bound attention phases and the
LayerNorm stat chains by interleaving independent GEMM work: each
projection is emitted as a generator of PSUM-group chunks, and the
cross-attention K/V projection of the current layer (computed from the
layer-invariant memory embeddings, resident in SBUF) is pulled chunk by
chunk into the AllGather gap, the SA attention stream, and the LN
windows. This avoids PE idle gaps that would re-throttle the HAM clock.

Embedding gather/pack/positional encodings are integer-indexed data
staging done on host; all FLOPs run on device.
"""
from collections import deque

import numpy as np

import concourse.bass as bass
import concourse.mybir as mybir
import concourse.tile as tile
from concourse import bacc, bass_utils

F32 = mybir.dt.float32
F32R = mybir.dt.float32r
BF16 = mybir.dt.float16  # fp16: FWL-eligible, 10-bit mantissa

B, D, H, HD, FF, L = 4, 1024, 16, 64, 4096, 6
TT, TA, ENR = 128, 512, 225
SEQ = TT + TA + ENR            # 865
TPAD = 896                     # 7 * 128
TH = 448                       # per-core half (padded)
PREFIX = TT + TA               # 640 = 5 * 128
NKT = TPAD // 128              # 7 key tiles
ND = D // 128                  # 8 feature tiles
VOCAB = 1024
EPS = 1e-5
NEG = -1e9
PW = 960                       # packed pair width: 448 | 64 gap | 448


# ---------------------------------------------------------------- host side

def sinusoidal_pe(T, d):
    pos = np.arange(T, dtype=np.float32)[:, None]
    div = np.exp(np.arange(0, d, 2, dtype=np.float32) * (-np.log(10000.0) / d))
    pe = np.zeros((T, d), dtype=np.float32)
    pe[:, 0::2] = np.sin(pos * div)
    pe[:, 1::2] = np.cos(pos * div)
    return pe


def host_embed(text, audio, enrolled_audio, text_len, audio_len,
               text_emb, audio_emb):
    """Replicates reference embed+pack. Returns [B, TPAD, D] f32 (pad zeros)."""
    te = text_emb[text] + sinusoidal_pe(TT, D)[None]        # [B,TT,D]
    ae = audio_emb[audio] + sinusoidal_pe(TA, D)[None]      # [B,TA,D]
    ee = audio_emb[enrolled_audio] + sinusoidal_pe(ENR, D)[None]
    out = np.zeros((B, TPAD, D), dtype=np.float32)
    for b in range(B):
        tl, al = int(text_len[b]), int(audio_len[b])
        out[b, :tl] = te[b, :tl]
        out[b, tl:tl + al] = ae[b, :al]
        out[b, tl + al:tl + al + ENR] = ee[b]
    return out


def host_masks(half):
    """Additive mask for SA key tiles 5,6 transposed: [256, TH]."""
    k = np.arange(PREFIX, PREFIX + 256)[:, None]            # 640..895
    q = half * TH + np.arange(TH)[None, :]
    blocked = (k > q) | (k >= SEQ)
    return np.where(blocked, NEG, 0.0).astype(np.float32)


def host_maskrep(half):
    """[2, 128, PW] pair-replicated masks (cols 0:448 and 512:960).
    fp16, so the blocked sentinel is -60000 (exp underflows to 0)."""
    m = host_masks(half)                                    # [256, TH]
    m = np.maximum(m, -60000.0)
    out = np.zeros((2, 128, PW), dtype=np.float16)
    for i in range(2):
        out[i, :, 0:TH] = m[i * 128:(i + 1) * 128]
        out[i, :, 512:512 + TH] = m[i * 128:(i + 1) * 128]
    return out


def host_kvalid6():
    k = PREFIX + 128 + np.arange(128)                       # 768..895
    return np.where(k < SEQ, 0.0, NEG).astype(np.float32)[:, None]


# ---------------------------------------------------------------- builder

def build_kernel(n_layers=L, skip_bv=False):
    nc = bacc.Bacc("TRN2", target_bir_lowering=False, debug=False,
                   num_devices=8)

    def din(name, shape, dt=F32R):
        return nc.dram_tensor(name, shape, dt, kind="ExternalInput")

    xT0_d = din("xT0", [D, TH])
    memT_d = din("memT", [D, TPAD], BF16)
    maskrep_d = din("maskrep", [2, 128, PW], BF16)
    kval6_d = din("kval6", [128, 1], F32)
    ones_col_d = din("ones_col", [128, 1])
    ones_r128_d = din("ones_r128", [1, 128])
    vones_d = din("vones", [128, H], BF16)
    k2sel_d = din("k2sel", [2, 128])
    neg_r448_d = din("neg_r448", [1, TH])

    sa_inT_d = din("sa_inT", [L, D, 3 * D], BF16)
    sa_outT_d = din("sa_outT", [L, D, D], BF16)
    ca_inT_d = din("ca_inT", [L, D, 3 * D], BF16)
    ca_outT_d = din("ca_outT", [L, D, D], BF16)
    ff1T_d = din("ff1T", [L, D, FF], BF16)
    ff2T_d = din("ff2T", [L, FF, D], BF16)
    outT_d = din("outT", [D, VOCAB], BF16)

    sa_inb_d = din("sa_inb", [L, 3 * D], F32)
    sa_outb_d = din("sa_outb", [L, D], F32)
    ca_inb_d = din("ca_inb", [L, 3 * D], F32)
    ca_outb_d = din("ca_outb", [L, D], F32)
    ff1b_d = din("ff1b", [L, FF], F32)
    ff2b_d = din("ff2b", [L, D], F32)
    outb_d = din("outb", [VOCAB], F32)
    lnw_d = [din(f"ln{i}w", [L, D], F32) for i in (1, 2, 3)]
    lnb_d = [din(f"ln{i}b", [L, D], F32) for i in (1, 2, 3)]

    yT_d = nc.dram_tensor("yT", [VOCAB, TH], F32, kind="ExternalOutput")

    uid = [0]

    def nm(p):
        uid[0] += 1
        return f"{p}_{uid[0]}"

    with tile.TileContext(nc) as tc:
        with (
            nc.allow_low_precision(reason="f32r compute; tol 2e-2"),
            tc.tile_pool(name="const", bufs=1) as constp,
            tc.tile_pool(name="xpool", bufs=8) as xpool,

            tc.tile_pool(name="tmpp", bufs=4) as tmpp,
            tc.tile_pool(name="rows", bufs=2) as rowp,
            tc.tile_pool(name="statp", bufs=2) as statp,
            tc.tile_pool(name="biasp", bufs=12) as biasp,
            tc.tile_pool(name="wglob", bufs=28) as wglob,
            tc.tile_pool(name="dram", bufs=2, space="DRAM") as dramp,
        ):
            # ---- constants
            ones_col = constp.tile([128, 1], F32R, name="ones_col")
            ones_r128 = constp.tile([1, 128], F32R, name="ones_r128")
            vones = constp.tile([128, H], BF16, name="vones")
            kval6 = constp.tile([128, 1], F32, name="kval6")
            mask5 = constp.tile([128, PW], BF16, name="mask5")
            mask6 = constp.tile([128, PW], BF16, name="mask6")
            k2sel_a = constp.tile([1, 128], F32R, name="k2sel_a")
            k2sel_b = constp.tile([1, 128], F32R, name="k2sel_b")
            lnrhs = constp.tile([2, TH], F32R, name="lnrhs")
            nc.sync.dma_start(out=ones_col[:], in_=ones_col_d.ap())
            nc.sync.dma_start(out=ones_r128[:], in_=ones_r128_d.ap())
            nc.sync.dma_start(out=vones[:], in_=vones_d.ap())
            nc.sync.dma_start(out=kval6[:], in_=kval6_d.ap())
            nc.sync.dma_start(out=mask5[:], in_=maskrep_d.ap()[0])
            nc.sync.dma_start(out=mask6[:], in_=maskrep_d.ap()[1])
            nc.sync.dma_start(out=k2sel_a[:], in_=k2sel_d.ap()[0:1, :])
            nc.sync.dma_start(out=k2sel_b[:], in_=k2sel_d.ap()[1:2, :])
            nc.sync.dma_start(out=lnrhs[1:2, :], in_=neg_r448_d.ap())
            eps_tile = constp.tile([1, 1], F32, name="eps_tile")
            nc.vector.memset(eps_tile[:], EPS)

            # ---- x tiles: fixed, updated in place through the whole net
            # (f32r master) plus bf16 shadows used as GEMM moving operands
            x_cur = []
            xb16 = []
            for t in range(ND):
                xt = xpool.tile([128, TH], F32R, name=nm("x"), tag="x")
                nc.sync.dma_start(out=xt[:],
                                  in_=xT0_d.ap()[t * 128:(t + 1) * 128, :])
                x_cur.append(xt)
                xb = xpool.tile([128, TH], BF16, name=nm("xb"), tag="xb")
                nc.vector.tensor_copy(xb[:], xt[:])
                xb16.append(xb)

            # ------------------------------------------- filler machinery
            pending = deque()

            def pull(n=1):
                k = 0
                while k < n and pending:
                    try:
                        next(pending[0])
                        k += 1
                    except StopIteration:
                        pending.popleft()

            def drain(g):
                for _ in g:
                    pass

            def drain_pooled(make_gen):
                """Run a generator to completion with its own 4-deep PSUM
                pool (dense phases; LIFO-safe because nothing else touches
                the pool stack during the drain)."""
                with tc.tile_pool(name=nm("dps"), bufs=4,
                                  space="PSUM") as p:
                    drain(make_gen(p))

            def drain_pending():
                while pending:
                    pull(1)

            # ---------------------------------------------------- helpers
            def load_bias_col(src_1d_ap, n, name):
                t = biasp.tile([128, n], F32, name=nm(name), tag="bcol")
                nc.sync.dma_start(
                    out=t[:], in_=src_1d_ap.rearrange("(c p) -> p c", p=128))
                return t

            def load_row(src_1d_ap, n, name):
                t = rowp.tile([1, n], F32R, name=nm(name), tag="row")
                nc.sync.dma_start(
                    out=t[:],
                    in_=src_1d_ap.rearrange("(a f) -> a f", a=1).bitcast(F32R))
                return t

            def gemm_chunks(ppool, wT2d, rhs_tiles, nout, evict, fdim=TH,
                            psbufs=4):
                """Generator: out^T[nout, fdim] = W @ rhs, yielding after
                each PSUM-group (one 128-row output tile)."""
                nk = len(rhs_tiles)
                for n0 in range(0, nout, 512):
                    w = min(512, nout - n0)
                    wts = []
                    for k in range(nk):
                        wt = wglob.tile([128, w], BF16, name=nm("w"),
                                        tag="w", bufs=16)
                        nc.sync.dma_start(
                            out=wt[:],
                            in_=wT2d[k * 128:(k + 1) * 128, n0:n0 + w])
                        wts.append(wt)
                    for m0 in range(0, w, 128):
                        ps = ppool.tile([128, 512], F32, name=nm("pg"),
                                        tag="pg", bufs=psbufs)
                        for k in range(nk):
                            nc.tensor.matmul(
                                ps[:, 0:fdim], wts[k][:, m0:m0 + 128],
                                rhs_tiles[k][:, :fdim],
                                start=(k == 0), stop=(k == nk - 1))
                        evict(n0 + m0, ps[:, 0:fdim])
                        yield

            def kv_chunks(ppool, inT2d, inb1d, src_tiles, kt, va,
                          psbufs=4):
                """Generator: fills kt [8 x (128, TPAD)] and
                va [7 x (128, H*65)] from src_tiles (8 x [128, TPAD])."""
                bk_col = load_bias_col(inb1d[D:2 * D], ND, "bk")
                bv_row = None if skip_bv else load_row(
                    inb1d[2 * D:3 * D], D, "bv")
                for t in range(NKT):
                    nc.sync.dma_start(
                        out=va[t][:].rearrange("p (h e) -> p h e", e=65)
                        [:, :, 64:65],
                        in_=vones_d.ap())
                if True:
                    for f0 in (0, TH):
                        for n0 in (0, 512):
                            wts = []
                            for k in range(ND):
                                wt = wglob.tile([128, 512], BF16,
                                                name=nm("w"), tag="w",
                                                bufs=16)
                                nc.sync.dma_start(
                                    out=wt[:],
                                    in_=inT2d[k * 128:(k + 1) * 128,
                                              D + n0:D + n0 + 512])
                                wts.append(wt)
                            for m0 in range(0, 512, 128):
                                ps = ppool.tile([128, 512], F32,
                                                name=nm("pk"), tag="pg",
                                                bufs=psbufs)
                                for k in range(ND):
                                    nc.tensor.matmul(
                                        ps[:, 0:TH], wts[k][:, m0:m0 + 128],
                                        src_tiles[k][:, f0:f0 + TH],
                                        start=(k == 0), stop=(k == ND - 1))
                                nc.vector.tensor_scalar_add(
                                    kt[(n0 + m0) // 128][:, f0:f0 + TH],
                                    ps[:, 0:TH],
                                    bk_col[:, (n0 + m0) // 128:
                                           (n0 + m0) // 128 + 1])
                                yield
                    for c0 in (0, 512):
                        wts = []
                        for k in range(ND):
                            wt = wglob.tile([128, 512], BF16, name=nm("wv"),
                                            tag="w", bufs=16)
                            nc.sync.dma_start(
                                out=wt[:],
                                in_=inT2d[k * 128:(k + 1) * 128,
                                          2 * D + c0:2 * D + c0 + 512])
                            wts.append(wt)
                        for t in range(NKT):
                            ps = ppool.tile([128, 512], F32,
                                            name=nm("pv"), tag="pg",
                                            bufs=psbufs)
                            for k in range(ND):
                                nc.tensor.matmul(
                                    ps[:],
                                    src_tiles[k][:, t * 128:(t + 1) * 128],
                                    wts[k][:], start=(k == 0),
                                    stop=(bv_row is None and k == ND - 1))
                            if bv_row is not None:
                                nc.tensor.matmul(ps[:], ones_r128[:, :128],
                                                 bv_row[:, c0:c0 + 512],
                                                 start=False, stop=True)
                            nc.vector.tensor_copy(
                                va[t][:].rearrange("p (h e) -> p h e", e=65)
                                [:, c0 // 64:c0 // 64 + 8, 0:64],
                                ps[:].rearrange("p (h e) -> p h e", e=64))
                            yield

            def layer_norm(x_tiles, w_col, w_row):
                """In-place post-norm LN over the feature (partition) dim."""
                with tc.tile_pool(name=nm("lnps"), bufs=2, space="PSUM") as lps:
                    mu_ps = lps.tile([1, TH], F32, name=nm("mups"), bufs=1)
                    s2_ps = lps.tile([1, TH], F32, name=nm("s2ps"), bufs=1)
                    for t in range(ND):
                        nc.tensor.matmul(mu_ps[:], ones_col[:],
                                         x_tiles[t][:],
                                         start=(t == 0), stop=(t == ND - 1))
                    for t in range(ND):
                        sq = tmpp.tile([128, TH], F32R, name=nm("sq"),
                                       tag="tmp")
                        nc.scalar.square(sq[:], x_tiles[t][:])
                        nc.tensor.matmul(s2_ps[:], ones_col[:], sq[:],
                                         start=(t == 0), stop=(t == ND - 1))
                    pull(2)
                    muex = statp.tile([1, 2 * TH], F32, name=nm("muex"),
                                      tag="st2", bufs=1)
                    mu = muex[0:1, 0:TH]
                    ex2 = muex[0:1, TH:2 * TH]
                    nc.scalar.activation(mu, mu_ps[:],
                                         mybir.ActivationFunctionType.Copy,
                                         scale=1.0 / D)
                    nc.scalar.activation(ex2, s2_ps[:],
                                         mybir.ActivationFunctionType.Copy,
                                         scale=1.0 / D)
                    var = statp.tile([1, TH], F32, name=nm("var"), tag="st")
                    nc.vector.tensor_tensor(var[:], mu, mu,
                                            mybir.AluOpType.mult)
                    nc.vector.tensor_tensor(var[:], ex2, var[:],
                                            mybir.AluOpType.subtract)
                    sd = statp.tile([1, TH], F32R, name=nm("sd"), tag="st")
                    nc.scalar.activation(sd[:], var[:],
                                         mybir.ActivationFunctionType.Sqrt,
                                         bias=eps_tile[:])
                    sdb_ps = lps.tile([128, TH], F32, name=nm("sdb"), bufs=1)
                    nc.tensor.matmul(sdb_ps[:], ones_r128[:], sd[:],
                                     start=True, stop=True)
                    pull(2)
                    rs_b = tmpp.tile([128, TH], F32, name=nm("rsb"),
                                     tag="rb", bufs=2)
                    nc.vector.reciprocal_approx_fast(out=rs_b[:],
                                                     in_=sdb_ps[:])
                    nc.vector.tensor_tensor(lnrhs[0:1, :], mu, rs_b[0:1, :],
                                            mybir.AluOpType.mult)
                    for t in range(ND):
                        aux = lps.tile([128, TH], F32, name=nm("aux"),
                                       tag="lnaux", bufs=2)
                        nc.tensor.matmul(aux[:],
                                         w_row[:, t * 128:(t + 1) * 128],
                                         lnrhs[:], start=True, stop=True)
                        t1 = tmpp.tile([128, TH], F32R, name=nm("t1"),
                                       tag="tmp")
                        nc.vector.tensor_tensor(t1[:], x_tiles[t][:],
                                                rs_b[:],
                                                mybir.AluOpType.mult)
                        nc.vector.scalar_tensor_tensor(
                            x_tiles[t][:], t1[:], w_col[:, t:t + 1], aux[:],
                            mybir.AluOpType.mult, mybir.AluOpType.subtract)
                        nc.vector.tensor_copy(xb16[t][:], x_tiles[t][:])

            def attention(pp, q_tiles, kt_tiles, vaug_tiles, masks, kval,
                          sbufs=1):
                """Returns attnT tiles (8 x [128, TH]) in pool pp.
                Pair-packed S/exp + PV one k-tile behind; pulls one filler
                chunk per k-tile step to keep the PE dense."""
                at = [pp.tile([128, TH], BF16, name=nm("at"), tag="attnT",
                              bufs=8) for _ in range(ND)]
                with (
                    tc.tile_pool(name=nm("aps"), bufs=sbufs,
                                 space="PSUM") as sps,
                    tc.tile_pool(name=nm("ops"), bufs=4, space="PSUM") as ops,
                ):
                    for h0 in range(0, H, 4):
                        quad = (h0, h0 + 1, h0 + 2, h0 + 3)
                        o_ps = {}
                        for hh in quad:
                            o_ps[hh] = ops.tile([65, TH], F32,
                                                name=nm("ops"), tag="po",
                                                bufs=4)
                        pbs = {}
                        for t in range(NKT + 1):
                            if t < NKT:
                                for pi in (0, 1):
                                    ha = h0 + 2 * pi
                                    ti = ha // 2
                                    ps = sps.tile([128, 1024], F32,
                                                  name=nm("sps"), tag="sst",
                                                  bufs=sbufs)
                                    nc.tensor.matmul(
                                        ps[:, 0:TH],
                                        kt_tiles[ti][0:64,
                                                     t * 128:(t + 1) * 128],
                                        q_tiles[ti][0:64, :],
                                        start=True, stop=True)
                                    nc.tensor.matmul(
                                        ps[:, 512:512 + TH],
                                        kt_tiles[ti][64:128,
                                                     t * 128:(t + 1) * 128],
                                        q_tiles[ti][64:128, :],
                                        start=True, stop=True)
                                    pb = tmpp.tile([128, PW], BF16,
                                                   name=nm("p"), tag="pexp",
                                                   bufs=3)
                                    if masks is not None and t >= 5:
                                        tm = tmpp.tile([128, PW], F32R,
                                                       name=nm("sm"),
                                                       tag="ptmp", bufs=1)
                                        nc.vector.tensor_tensor(
                                            tm[:], ps[:, 0:PW],
                                            masks[t - 5][:],
                                            mybir.AluOpType.add)
                                        nc.scalar.activation(
                                            pb[:], tm[:],
                                            mybir.ActivationFunctionType.Exp)
                                    elif kval is not None and t == NKT - 1:
                                        nc.scalar.activation(
                                            pb[:], ps[:, 0:PW],
                                            mybir.ActivationFunctionType.Exp,
                                            bias=kval[:])
                                    else:
                                        nc.scalar.activation(
                                            pb[:], ps[:, 0:PW],
                                            mybir.ActivationFunctionType.Exp)
                                    pbs[(pi, t)] = pb
                            if t > 0:
                                for pi in (0, 1):
                                    ha = h0 + 2 * pi
                                    pb = pbs.pop((pi, t - 1))
                                    for s, hh in ((0, ha), (512, ha + 1)):
                                        nc.tensor.matmul(
                                            o_ps[hh][:],
                                            vaug_tiles[t - 1][:].rearrange(
                                                "p (h e) -> p h e",
                                                e=65)[:, hh, :],
                                            pb[:, s:s + TH],
                                            start=(t - 1 == 0),
                                            stop=(t - 1 == NKT - 1))
                            pull(1)
                        for pi in (0, 1):
                            ha = h0 + 2 * pi
                            dena = statp.tile([1, TH], F32R, name=nm("dna"),
                                              tag="stda", bufs=2)
                            denb = statp.tile([1, TH], F32R, name=nm("dnb"),
                                              tag="stdb", bufs=2)
                            nc.vector.tensor_copy(dena[:],
                                                  o_ps[ha][64:65, :])
                            nc.vector.tensor_copy(denb[:],
                                                  o_ps[ha + 1][64:65, :])
                            r_ps = sps.tile([128, 1024], F32,
                                            name=nm("rps"), tag="sst",
                                            bufs=sbufs)
                            nc.tensor.matmul(r_ps[:, 0:TH],
                                             k2sel_a[:], dena[:],
                                             start=True, stop=False)
                            nc.tensor.matmul(r_ps[:, 0:TH],
                                             k2sel_b[:], denb[:],
                                             start=False, stop=True)
                            rb = tmpp.tile([128, TH], F32, name=nm("rb"),
                                           tag="rb", bufs=2)
                            nc.vector.reciprocal_approx_fast(
                                out=rb[:], in_=r_ps[:, 0:TH])
                            ti = ha // 2
                            nc.vector.tensor_tensor(
                                at[ti][0:64, :], o_ps[ha][0:64, :],
                                rb[0:64, :], mybir.AluOpType.mult)
                            nc.vector.tensor_tensor(
                                at[ti][64:128, :], o_ps[ha + 1][0:64, :],
                                rb[64:128, :], mybir.AluOpType.mult)
                return at

            def qproj_gen(pp, inT2d, inb1d):
                q_t = [pp.tile([128, TH], BF16, name=nm("q"), tag="q",
                               bufs=8) for _ in range(ND)]
                bq_col = load_bias_col(inb1d[0:D], ND, "bq")

                def ev_q(n0, ps):
                    nc.vector.tensor_scalar_add(
                        q_t[n0 // 128][:], ps,
                        bq_col[:, n0 // 128:n0 // 128 + 1])
                return q_t, ev_q

            def out_proj(wT2d, b1d, at):
                bo_col = load_bias_col(b1d, ND, "bo")

                def ev_o(n0, ps):
                    t = n0 // 128
                    nc.vector.scalar_tensor_tensor(
                        x_cur[t][:], ps, bo_col[:, t:t + 1], x_cur[t][:],
                        mybir.AluOpType.add, mybir.AluOpType.add)
                drain_pooled(lambda p: gemm_chunks(p, wT2d, at, D, ev_o))

            def do_ln(idx, l):
                lwb = rowp.tile([2, D], F32R, name=nm(f"ln{idx}wb"),
                                tag="row")
                nc.sync.dma_start(
                    out=lwb[0:1, :],
                    in_=lnw_d[idx].ap()[l].rearrange(
                        "(a f) -> a f", a=1).bitcast(F32R))
                nc.sync.dma_start(
                    out=lwb[1:2, :],
                    in_=lnb_d[idx].ap()[l].rearrange(
                        "(a f) -> a f", a=1).bitcast(F32R))
                lwc = load_bias_col(lnw_d[idx].ap()[l], ND, f"ln{idx}wc")
                layer_norm(x_cur, lwc, lwb)

            # ---------------------------------------------------- layers
            for l in range(n_layers):
                # CA K/V of this layer: computed from the (re-loaded) memory
                # embeddings, consumed as filler during AG gap / SA
                # attention / LNs. Lives in its own pool spanning the layer.
                kvp_ctx = tc.tile_pool(name=nm("kvl"), bufs=2)
                kvp = kvp_ctx.__enter__()
                memt = [kvp.tile([128, TPAD], BF16, name=nm("memt"),
                                 tag="mem", bufs=ND) for _ in range(ND)]
                for t in range(ND):
                    nc.sync.dma_start(
                        out=memt[t][:],
                        in_=memT_d.ap()[t * 128:(t + 1) * 128, :])
                kt_ca = [kvp.tile([128, TPAD], BF16, name=nm("ktca"),
                                  tag="ktca", bufs=ND) for _ in range(ND)]
                va_ca = [kvp.tile([128, H * 65], BF16, name=nm("vaca"),
                                  tag="vaca", bufs=NKT) for _ in range(NKT)]
                fill_ctx = tc.tile_pool(name=nm("fillps"), bufs=2,
                                        space="PSUM")
                fillps = fill_ctx.__enter__()
                pending.append(kv_chunks(fillps, ca_inT_d.ap()[l],
                                         ca_inb_d.ap()[l],
                                         memt, kt_ca, va_ca, psbufs=2))

                ag_in = dramp.tile([D, TH], BF16, name=nm("agin"), tag="agi")
                ag_out = dramp.tile([2 * D, TH], BF16, name=nm("agout"),
                                    tag="ago")
                for t in range(ND):
                    nc.sync.dma_start(
                        out=ag_in[t * 128:(t + 1) * 128, :], in_=xb16[t][:])
                nc.gpsimd.collective_compute(
                    "AllGather", mybir.AluOpType.bypass,
                    replica_groups=[[0, 1], [2, 3], [4, 5], [6, 7]],
                    ins=[ag_in[:].opt()], outs=[ag_out[:].opt()])

                with tc.tile_pool(name=nm("attl"), bufs=2) as attl:
                    # fill the AllGather gap: SA Q proj (own half) + CA K/V
                    q_sa, ev_qsa = qproj_gen(attl, sa_inT_d.ap()[l],
                                             sa_inb_d.ap()[l])
                    drain_pooled(lambda p: gemm_chunks(
                        p, sa_inT_d.ap()[l][:, 0:D], xb16, D, ev_qsa))
                    pull(14)

                    # SA K/V over the gathered full sequence
                    xfull = [attl.tile([128, TPAD], BF16, name=nm("xf"),
                                       tag="xfull", bufs=8)
                             for _ in range(ND)]
                    for t in range(ND):
                        nc.sync.dma_start(
                            out=xfull[t][:, 0:TH],
                            in_=ag_out[t * 128:(t + 1) * 128, :])
                        nc.sync.dma_start(
                            out=xfull[t][:, TH:TPAD],
                            in_=ag_out[D + t * 128:D + (t + 1) * 128, :])
                    kt_sa = [attl.tile([128, TPAD], BF16, name=nm("ktsa"),
                                       tag="ktsa", bufs=ND)
                             for _ in range(ND)]
                    va_sa = [attl.tile([128, H * 65], BF16, name=nm("vasa"),
                                       tag="vasa", bufs=NKT)
                             for _ in range(NKT)]
                    drain_pooled(lambda p: kv_chunks(
                        p, sa_inT_d.ap()[l], sa_inb_d.ap()[l],
                        xfull, kt_sa, va_sa))

                    at = attention(attl, q_sa, kt_sa, va_sa,
                                   (mask5, mask6), None)
                    out_proj(sa_outT_d.ap()[l], sa_outb_d.ap()[l], at)
                    do_ln(0, l)

                q_ca, ev_qca = qproj_gen(kvp, ca_inT_d.ap()[l],
                                         ca_inb_d.ap()[l])
                drain_pooled(lambda p: gemm_chunks(
                    p, ca_inT_d.ap()[l][:, 0:D], xb16, D, ev_qca))
                drain_pending()
                fill_ctx.__exit__(None, None, None)
                at = attention(kvp, q_ca, kt_ca, va_ca, None, kval6,
                               sbufs=2)
                out_proj(ca_outT_d.ap()[l], ca_outb_d.ap()[l], at)
                do_ln(1, l)

                # ================= FFN =================
                with tc.tile_pool(name=nm("ffl"), bufs=2) as ffp:
                    ht = [ffp.tile([128, TH], BF16, name=nm("h"), tag="h",
                                   bufs=FF // 128) for _ in range(FF // 128)]
                    b1_col = load_bias_col(ff1b_d.ap()[l], FF // 128, "b1")

                    def ev_h(n0, ps):
                        t = n0 // 128
                        nc.vector.tensor_scalar(
                            ht[t][:], ps, b1_col[:, t:t + 1], 0.0,
                            mybir.AluOpType.add, mybir.AluOpType.max)
                    drain_pooled(lambda p: gemm_chunks(
                        p, ff1T_d.ap()[l], xb16, FF, ev_h))

                    b2_col = load_bias_col(ff2b_d.ap()[l], ND, "b2")

                    def ev_f(n0, ps):
                        # first half adds the bias, second half adds plain
                        t = n0 // 128
                        nc.vector.scalar_tensor_tensor(
                            x_cur[t][:], ps, b2_col[:, t:t + 1], x_cur[t][:],
                            mybir.AluOpType.add, mybir.AluOpType.add)

                    def ev_f2(n0, ps):
                        t = n0 // 128
                        nc.vector.tensor_tensor(
                            x_cur[t][:], ps, x_cur[t][:],
                            mybir.AluOpType.add)
                    drain_pooled(lambda p: gemm_chunks(
                        p, ff2T_d.ap()[l][0:FF // 2, :], ht[:16], D, ev_f))
                    drain_pooled(lambda p: gemm_chunks(
                        p, ff2T_d.ap()[l][FF // 2:FF, :], ht[16:], D,
                        ev_f2))
                    do_ln(2, l)
                kvp_ctx.__exit__(None, None, None)

            # ---- final projection
            ob_col = load_bias_col(outb_d.ap(), VOCAB // 128, "ob")

            def ev_y(n0, ps):
                y = tmpp.tile([128, TH], F32, name=nm("y"), tag="tmp")
                nc.vector.tensor_scalar_add(
                    y[:], ps, ob_col[:, n0 // 128:n0 // 128 + 1])
                nc.sync.dma_start(out=yT_d.ap()[n0:n0 + 128, :], in_=y[:])
            drain_pooled(lambda p: gemm_chunks(
                p, outT_d.ap(), xb16, VOCAB, ev_y))

    nc.compile()
    return nc


# ---------------------------------------------------------------- wrapper

def prep_in_maps(inputs):
    f32 = lambda a: np.ascontiguousarray(np.asarray(a, dtype=np.float32))
    embed = host_embed(
        np.asarray(inputs["text"]), np.asarray(inputs["audio"]),
        np.asarray(inputs["enrolled_audio"]),
        np.asarray(inputs["text_len_batch"]),
        np.asarray(inputs["audio_len_batch"]),
        f32(inputs["text_emb"]), f32(inputs["audio_emb"]))
    embT = np.ascontiguousarray(embed.transpose(0, 2, 1))   # [B, D, TPAD]

    bf = lambda a: np.ascontiguousarray(a.astype(np.float16))
    tr = lambda a: np.ascontiguousarray(
        np.asarray(a, dtype=np.float32).transpose(0, 2, 1))
    sa_inT = tr(inputs["sa_in_w"])      # [L, D, 3D]
    ca_inT = tr(inputs["ca_in_w"])
    sa_inT[:, :, :D] *= 0.125           # fold 1/sqrt(hd) into Q
    ca_inT[:, :, :D] *= 0.125
    sa_inb = f32(inputs["sa_in_b"]).copy()
    ca_inb = f32(inputs["ca_in_b"]).copy()
    sa_inb[:, :D] *= 0.125
    ca_inb[:, :D] *= 0.125

    shared = dict(
        kval6=host_kvalid6(),
        ones_col=np.ones((128, 1), np.float32),
        ones_r128=np.ones((1, 128), np.float32),
        vones=np.ones((128, H), np.float16),
        neg_r448=np.full((1, 448), -1.0, np.float32),
        k2sel=np.concatenate([
            np.concatenate([np.ones((1, 64)), np.zeros((1, 64))], 1),
            np.concatenate([np.zeros((1, 64)), np.ones((1, 64))], 1),
        ]).astype(np.float32),
        sa_inT=bf(sa_inT), sa_outT=bf(tr(inputs["sa_out_w"])),
        ca_inT=bf(ca_inT), ca_outT=bf(tr(inputs["ca_out_w"])),
        ff1T=bf(tr(inputs["ff1_w"])), ff2T=bf(tr(inputs["ff2_w"])),
        outT=bf(np.ascontiguousarray(f32(inputs["out_w"]).T)),
        sa_inb=sa_inb, sa_outb=f32(inputs["sa_out_b"]),
        ca_inb=ca_inb, ca_outb=f32(inputs["ca_out_b"]),
        ff1b=f32(inputs["ff1_b"]), ff2b=f32(inputs["ff2_b"]),
        outb=f32(inputs["out_b"]),
        ln1w=f32(inputs["ln1_w"]), ln1b=f32(inputs["ln1_b"]),
        ln2w=f32(inputs["ln2_w"]), ln2b=f32(inputs["ln2_b"]),
        ln3w=f32(inputs["ln3_w"]), ln3b=f32(inputs["ln3_b"]),
    )
    in_maps = []
    for c in range(8):
        bb, hh = c // 2, c % 2
        m = dict(shared)
        m["xT0"] = np.ascontiguousarray(embT[bb][:, hh * TH:(hh + 1) * TH])
        m["memT"] = bf(embT[bb])
        m["maskrep"] = host_maskrep(hh)
        in_maps.append(m)
    return in_maps


_NC_CACHE = {}


def run(inputs, n_layers=L, trace=False):
    skip_bv = (not np.any(np.asarray(inputs["sa_in_b"])[:, 2 * D:])
               and not np.any(np.asarray(inputs["ca_in_b"])[:, 2 * D:]))
    key = (n_layers, skip_bv)
    if key not in _NC_CACHE:
        _NC_CACHE[key] = build_kernel(n_layers, skip_bv)
    nc = _NC_CACHE[key]
    in_maps = prep_in_maps(inputs)
    res = bass_utils.run_bass_kernel_spmd(
        nc, in_maps, core_ids=list(range(8)), trace=trace)
    out = np.zeros((B, SEQ, VOCAB), dtype=np.float32)
    for c in range(8):
        bb, hh = c // 2, c % 2
        cols = TH if hh == 0 else SEQ - TH
        out[bb, hh * TH:hh * TH + cols, :] = \
            res.results[c]["yT"][:, :cols].T
    return out, res


def kernel(**inputs):
    out, _ = run(inputs)
    return out
